# revision 13
# baseline (speedup 1.0000x reference)
"""Trainium2 Bass kernel for a Bahdanau-style attention module.

Reference computation (B=64, S=512, H=1000, D=2H=2000):
    ci   = context @ W_a.T                        # (B,S,H)
    hi   = decoder_hidden @ W_b.T                 # (1,B,H) -> (B,1,H)
    att  = tanh(ci + hi) @ W_c_w.T + W_c_b        # (B,S,1)
    att  = where(mask, -1e6, att); att = softmax(att, axis=1)
    ws   = att.T @ context                        # (B,1,2H)
    out  = ws @ dense_w.T + dense_b               # (B,1,H)

Strategy: data-parallel over batch across 8 NeuronCores (8 batch slots
per core, weights replicated; no collectives).  Host-side prep:
  * masked source positions are compacted away (their softmax weight is
    exactly 0), batches are sorted by unmasked count and assigned so
    that slot k holds ranks [8k, 8k+8) -- every core's slot-k matmuls
    then share one tight width SC_k instead of a global worst-case pad;
  * hidden_in = decoder_hidden @ W_b.T is computed on host (tiny GEMM)
    and shipped as the tanh bias directly;
  * all tensors are pre-cast (ctx/W_a fp8 for the DoubleRow score path,
    bf16 for the value path) and packed partition-major so every DMA is
    one contiguous 128-partition transfer.
On-chip: fp8 DoubleRow matmuls for ci, bf16 for scores/ws/dense;
softmax normalization is deferred past the weighted sum (ws accumulates
raw exp() and is rescaled once by 1/sum per batch row); the dense tail
runs as 4 concurrent PE column-quadrants.
"""

import numpy as np
import ml_dtypes

import concourse.bass as bass
import concourse.tile as tile
from concourse import bacc, mybir
from concourse.bass_utils import run_bass_kernel_spmd
from concourse.masks import make_identity

BF16 = ml_dtypes.bfloat16
FP8 = ml_dtypes.float8_e4m3
WA_SCALE = 64.0

B = 64          # global batch
BC = 8          # batch slots per core
NCORES = 8
S = 512         # encoder source length
H = 1000
HP = 1024       # padded hidden
D = 2000
DP = 2048       # padded 2*hidden
KD = DP // 128  # 16 k-tiles over padded contraction dim
KH = HP // 128  # 8 h-tiles
F32 = mybir.dt.float32
BF = mybir.dt.bfloat16
F8 = mybir.dt.float8e4

N_WARM = 3      # PE warmup matmuls (data-arrival jitter buffer)


def _build_graph(scs, kscs):
    """scs[k] = compacted source width of slot k (multiple of 32, sorted
    descending); kscs[k] = number of 128-row s-tiles for the ws path."""
    nc = bacc.Bacc()

    t_off = np.cumsum([0] + [KD * sc for sc in scs])
    n_off = np.cumsum([0] + [ksc * DP for ksc in kscs])
    m_off = np.cumsum([0] + list(scs))

    ctxT = nc.declare_dram_parameter("ctxT", [128, int(t_off[-1])], F8, isOutput=False)
    ctxN = nc.declare_dram_parameter("ctxN", [128, int(n_off[-1])], BF, isOutput=False)
    waT = nc.declare_dram_parameter("waT", [128, KH, KD, 128], F8, isOutput=False)
    dwT = nc.declare_dram_parameter("dwT", [128, KD * HP], BF, isOutput=False)
    hidT = nc.declare_dram_parameter("hidT", [128, KH * BC], F32, isOutput=False)
    wcT = nc.declare_dram_parameter("wcT", [128, KH], BF, isOutput=False)
    maskv = nc.declare_dram_parameter("maskv", [1, int(m_off[-1])], F32, isOutput=False)
    dbias = nc.declare_dram_parameter("dbias", [128, 256], F32, isOutput=False)
    out_ext = nc.declare_dram_parameter("out", [128, 256], F32, isOutput=True)

    with tile.TileContext(nc) as tc:
        with (
            tc.tile_pool(name="const", bufs=1) as cpool,
            tc.tile_pool(name="ctxTp", bufs=3) as ctxT_pool,
            tc.tile_pool(name="ctxNp", bufs=2) as ctxN_pool,
            tc.tile_pool(name="tanhp", bufs=9) as tanh_pool,
            tc.tile_pool(name="smallp", bufs=2) as small_pool,
            tc.tile_pool(name="attp", bufs=2) as att_pool,
            tc.tile_pool(name="ci", bufs=3, space="PSUM") as ci_pool,
            tc.tile_pool(name="scps", bufs=3, space="PSUM") as sc_pool,
            tc.tile_pool(name="wsps", bufs=1, space="PSUM") as ws_pool,
            tc.tile_pool(name="accps", bufs=1, space="PSUM") as acc_pool,
        ):
            # ---- PE warmup: very first ops in the graph -----------------
            # Startup is DMA-bound and the HAM clock-gate needs ~3.4us of
            # PE activity to reach 2.4 GHz.  DVE memset (fast engine
            # start) rather than gpsimd so the first matmul issues early.
            warm_sb = cpool.tile([128, 512], BF, tag="warm")
            nc.vector.memset(warm_sb[:], 0.0)
            warm_ps = acc_pool.tile([128, 512], F32, tag="acc")
            for _w in range(N_WARM):
                nc.tensor.matmul(
                    warm_ps[:],
                    warm_sb[:, 0:128],
                    warm_sb[:],
                    start=True,
                    stop=True,
                    skip_group_check=True,
                )
            warm_out = cpool.tile([1, 16], F32, tag="warmout")
            nc.vector.tensor_copy(warm_out[:], warm_ps[0:1, 0:16])
            act_prime = cpool.tile([1, 1], BF, tag="actprime")
            nc.scalar.activation(
                act_prime[:], warm_sb[0:1, 0:1],
                mybir.ActivationFunctionType.Tanh,
            )

            # ---- resident weights / constants -------------------------------
            # Startup-critical DMA order: slot-0 context + the first W_a
            # h-chunk unblock the first ci matmuls; hidT must beat the
            # first tanh; slot-1 context prefetches right behind W_a.
            ctxT_tiles = {}

            def issue_ctxT(b, gate_src=None):
                t = ctxT_pool.tile([128, KD, scs[b]], F8, tag="ctxT")
                if gate_src is not None:
                    nc.vector.tensor_copy(t[0:1, 0, 0:1], gate_src)
                nc.sync.dma_start(t[:], ctxT[:, int(t_off[b]) : int(t_off[b + 1])])
                ctxT_tiles[b] = t

            # In-flight DMAs share the HBM pipes round-robin, so an
            # unconstrained preamble starves the first ci matmul's inputs
            # (waT h0 + ctxT slot 0).  Every other early transfer is
            # WAW-gated: a 1-element DVE copy into its destination that
            # reads a predecessor's landed data, so the chain streams
            # in exactly the order the compute consumes it.
            waT_sb = cpool.tile([128, KH, KD, 128], F8, tag="waT")
            nc.sync.dma_start(waT_sb[:, 0], waT[:, 0])
            issue_ctxT(0)
            ctxT0_t = ctxT_tiles[0]

            def gate(dst_1elem, src_1elem):
                nc.vector.tensor_copy(dst_1elem, src_1elem)

            hidT_sb = cpool.tile([128, KH * BC], F32, tag="hidT")
            gate(hidT_sb[0:1, 0:1], waT_sb[0:1, 0, 0, 0:1])
            nc.scalar.dma_start(hidT_sb[:], hidT[:])
            for h in range(1, KH):
                prev = ctxT0_t[0:1, 0, 0:1] if h == 1                     else waT_sb[0:1, h - 1, 0, 0:1]
                gate(waT_sb[0:1, h, 0, 0:1], prev)
                nc.scalar.dma_start(waT_sb[:, h], waT[:, h])
            issue_ctxT(1, gate_src=waT_sb[0:1, 3, 0, 0:1])
            wcT_sb = cpool.tile([128, KH], BF, tag="wcT")
            gate(wcT_sb[0:1, 0:1], waT_sb[0:1, 4, 0, 0:1])
            nc.scalar.dma_start(wcT_sb[:], wcT[:])
            maskv_sb = cpool.tile([1, int(m_off[-1])], F32, tag="maskv")
            gate(maskv_sb[0:1, 0:1], waT_sb[0:1, 4, 0, 0:1])
            nc.scalar.dma_start(maskv_sb[:], maskv[:])
            dwT_sb = cpool.tile([128, KD * HP], BF, tag="dwT")
            dbias_sb = cpool.tile([128, 256], F32, tag="dbias")

            ident_b = cpool.tile([128, 128], BF, tag="identb")
            make_identity(nc, ident_b[:])
            one_f = cpool.tile([1, 1], F32, tag="onef")
            nc.gpsimd.memset(one_f[:], 1.0)

            # 1/sum(exp) per slot, replicated to columns {b, 32+b, 64+b,
            # 96+b} at the tail so one PE transpose yields the per-row
            # rescale vector for the ws PSUM.
            invrow = cpool.tile([1, 128], F32, tag="invrow")
            wsT_sb = cpool.tile([128, KD * BC], BF, tag="wsT")

            # ---- main pipeline over batch slots -----------------------------
            ctxN_tiles = [None] * BC
            exp_tiles = [None] * BC
            ws_psum = ws_pool.tile([128, 512], F32, tag="wsps")

            def stage_scores(b):
                """ci matmuls + tanh + scores + masked exp for slot b."""
                sc = scs[b]
                if b + 1 < BC and b >= 1:
                    issue_ctxT(b + 1)
                ctxT_t = ctxT_tiles.pop(b)
                ctxN_t = ctxN_pool.tile([128, kscs[b] * DP], BF, tag="ctxN")
                if b == 0:
                    gate(ctxN_t[0:1, 0:1], waT_sb[0:1, 5, 0, 0:1])
                nc.sync.dma_start(
                    ctxN_t[:], ctxN[:, int(n_off[b]) : int(n_off[b + 1])]
                )
                ctxN_tiles[b] = ctxN_t
                if 2 <= b <= 6:
                    # tail-only data, streamed mid-loop in 5 chunks
                    c5 = b - 2
                    seg = KD * HP // 5
                    lo = c5 * seg
                    hi = KD * HP if c5 == 4 else (c5 + 1) * seg
                    nc.sync.dma_start(dwT_sb[:, lo:hi], dwT[:, lo:hi])
                if b == 6:
                    nc.sync.dma_start(dbias_sb[:], dbias[:])

                psum_sc = sc_pool.tile([1, sc], F32, tag="sc")
                tanh_tiles = {}
                ci_tiles = {}

                def emit_ci(h):
                    psum_ci = ci_pool.tile([128, sc], F32, tag="ci")
                    for g in range(KD // 2):
                        nc.tensor.matmul(
                            psum_ci[:],
                            waT_sb[:, h, 2 * g : 2 * g + 2, :],
                            ctxT_t[:, 2 * g : 2 * g + 2, :],
                            start=(g == 0),
                            stop=(g == KD // 2 - 1),
                            perf_mode=mybir.MatmulPerfMode.DoubleRow,
                        )
                    ci_tiles[h] = psum_ci

                def emit_tanh(h):
                    tanh_t = tanh_pool.tile([128, sc], BF, tag="tanh")
                    nc.scalar.activation(
                        tanh_t[:],
                        ci_tiles.pop(h)[:],
                        mybir.ActivationFunctionType.Tanh,
                        bias=hidT_sb[:, h * BC + b : h * BC + b + 1],
                        scale=1.0 / WA_SCALE,
                    )
                    tanh_tiles[h] = tanh_t

                # tanh lags ci by `lag` h-blocks so the PE never waits on
                # ACT; all scores matmuls go after the ci blocks so the
                # big-matmul pipeline breaks once per slot.
                lag = 2 if b == 0 else 1
                for h in range(KH):
                    emit_ci(h)
                    if h >= lag:
                        emit_tanh(h - lag)
                for h in range(KH - lag, KH):
                    emit_tanh(h)
                for h in range(KH):
                    nc.tensor.matmul(
                        psum_sc[:],
                        wcT_sb[:, h : h + 1],
                        tanh_tiles.pop(h)[:],
                        start=(h == 0),
                        stop=(h == KH - 1),
                    )

                # masked exp on a single partition.  No max-subtraction:
                # scores are O(1) and masked entries are -1e6 (exp -> 0).
                # Normalization is deferred: ws accumulates raw exp and is
                # rescaled by 1/esum at the tail.
                sc_sb = small_pool.tile([1, sc], F32, tag="scsb")
                nc.vector.tensor_tensor(
                    sc_sb[:], psum_sc[:],
                    maskv_sb[0:1, int(m_off[b]) : int(m_off[b + 1])],
                    op=mybir.AluOpType.add,
                )
                exp_sb = small_pool.tile([1, sc], BF, tag="exp")
                esum = small_pool.tile([1, 1], F32, tag="esum")
                nc.scalar.activation(
                    exp_sb[:], sc_sb[:], mybir.ActivationFunctionType.Exp,
                    bias=0.0, scale=1.0, accum_out=esum[:],
                )
                nc.vector.reciprocal(invrow[0:1, b : b + 1], esum[:])
                exp_tiles[b] = exp_sb

            def stage_ws(b):
                """exp transpose + weighted sum for slot b (accumulates into
                the persistent ws psum; rows j != b add exactly zero because
                attT_b is zero outside column b)."""
                sc = scs[b]
                ksc = kscs[b]
                exp_sb = exp_tiles[b]
                attT_b = att_pool.tile([128, kscs[0] * BC], BF, tag="attTb")
                nc.gpsimd.memset(attT_b[:], 0.0)
                for st in range(ksc):
                    w = min(128, sc - st * 128)
                    pt = sc_pool.tile([128, 1], BF, tag="sc")
                    nc.tensor.transpose(
                        pt[0:w, :], exp_sb[0:1, st * 128 : st * 128 + w],
                        ident_b[0:1, 0:1],
                    )
                    nc.vector.tensor_copy(
                        attT_b[0:w, st * BC + b : st * BC + b + 1],
                        pt[0:w, :],
                    )
                ctxN_t = ctxN_tiles[b]
                for st in range(ksc):
                    for nch in range(4):
                        nc.tensor.matmul(
                            ws_psum[32 * nch : 32 * nch + BC, :],
                            attT_b[:, st * BC : (st + 1) * BC],
                            ctxN_t[:, st * DP + nch * 512 : st * DP + (nch + 1) * 512],
                            start=(b == 0 and st == 0),
                            stop=(b == BC - 1 and st == ksc - 1),
                            tile_position=(0, 32 * nch),
                            skip_group_check=True,
                        )

            # software pipeline: scores(b) runs while ws(b-1) consumes
            for b in range(BC + 1):
                if b < BC:
                    stage_scores(b)
                if b == BC:
                    # einv prep off the tail critical path: only needs the
                    # slot-7 reciprocal, so it hides under ws(6)/ws(7)
                    for n in range(1, 4):
                        nc.vector.tensor_copy(
                            invrow[0:1, 32 * n : 32 * n + BC], invrow[0:1, 0:BC]
                        )
                    einv_ps = sc_pool.tile([128, 1], F32, tag="sc")
                    nc.tensor.transpose(einv_ps[:], invrow[:], one_f[:])
                    einv_sb = cpool.tile([128, 1], F32, tag="einvsb")
                    nc.vector.tensor_copy(einv_sb[:], einv_ps[:])
                if b >= 1:
                    stage_ws(b - 1)

            # ---- tail: normalize ws, transpose, dense layer -----------------
            ws_col = cpool.tile([128, 512], BF, tag="wscol")
            nc.vector.tensor_scalar_mul(ws_col[:], ws_psum[:], einv_sb[:])
            # consecutive transposes hit different PE row-groups so they
            # overlap in the array
            for k in [4 * q + j for j in range(4) for q in range(4)]:
                nch, kk = divmod(k, 4)
                pt = sc_pool.tile([128, BC], BF, tag="sc")
                nc.tensor.transpose(
                    pt[:],
                    ws_col[32 * nch : 32 * nch + BC, kk * 128 : (kk + 1) * 128],
                    ident_b[32 * nch : 32 * nch + BC, 32 * nch : 32 * nch + BC],
                    tile_position=(32 * nch, 0),
                )
                nc.vector.tensor_copy(wsT_sb[:, k * BC : (k + 1) * BC], pt[:])

            # dense: 4 concurrent column-quadrants, h-quarter q -> rows 32q+b
            psum_d = acc_pool.tile([128, 256], F32, tag="acc")
            for k in range(KD):
                for q in range(4):
                    nc.tensor.matmul(
                        psum_d[32 * q : 32 * q + BC, :],
                        wsT_sb[:, k * BC : (k + 1) * BC],
                        dwT_sb[:, k * HP + q * 256 : k * HP + (q + 1) * 256],
                        start=(k == 0),
                        stop=(k == KD - 1),
                        tile_position=(0, 32 * q),
                        skip_group_check=True,
                    )
            out_sb = cpool.tile([128, 256], F32, tag="outsb")
            nc.vector.tensor_tensor(
                out_sb[:], psum_d[:], dbias_sb[:], op=mybir.AluOpType.add
            )
            nc.sync.dma_start(out_ext[:], out_sb[:])

    nc.compile()
    return nc


_GRAPH = None
_GRAPH_KEY = None


def _plan(mask):
    """Sorted slot assignment + per-slot compact widths."""
    n = (~mask[:, :, 0]).sum(axis=1).astype(np.int64)
    order = np.argsort(-n, kind="stable")
    scs, kscs = [], []
    for k in range(BC):
        m = int(n[order[k * NCORES]])
        sc = max(32, -(-m // 4) * 4)
        scs.append(sc)
        kscs.append(-(-sc // 128))
    return n, order, scs, kscs


def _prep_inputs(decoder_hidden, context, mask, W_a, W_b, W_c_w, W_c_b,
                 dense_w, dense_b, n, order, scs, kscs):
    """Shard + pad + cast + pack all inputs into per-core input maps."""
    t_off = np.cumsum([0] + [KD * sc for sc in scs])
    n_off = np.cumsum([0] + [ksc * DP for ksc in kscs])
    m_off = np.cumsum([0] + list(scs))

    # W_a, h-major fp8: [p, h, kd, c] = W_a.T[kd*128+p, h*128+c] * WA_SCALE
    wa = np.zeros((DP, HP), dtype=np.float32)
    wa[:D, :H] = W_a.T.astype(np.float32) * WA_SCALE
    waT_p = np.ascontiguousarray(
        wa.reshape(KD, 128, KH, 128).transpose(1, 2, 0, 3)
    ).astype(FP8)

    dw = np.zeros((DP, HP), dtype=BF16)
    dw[:D, :H] = dense_w.T.astype(BF16)
    dwT_p = np.ascontiguousarray(
        dw.reshape(KD, 128, HP).transpose(1, 0, 2).reshape(128, KD * HP)
    )
    wc = np.zeros((HP,), dtype=np.float32)
    wc[:H] = W_c_w[0].astype(np.float32)
    wcT_p = np.ascontiguousarray(wc.reshape(KH, 128).T.astype(BF16))

    db = np.zeros((HP,), dtype=np.float32)
    db[:H] = dense_b.astype(np.float32)
    dbias_p = np.zeros((128, 256), dtype=np.float32)
    for q in range(4):
        dbias_p[32 * q : 32 * q + BC, :] = db[q * 256 : (q + 1) * 256]

    # hidden_in on host (tiny GEMM), padded + partition-major
    hi = decoder_hidden[0].astype(np.float32) @ W_b.T.astype(np.float32)  # (B, H)
    hip = np.zeros((B, HP), dtype=np.float32)
    hip[:, :H] = hi

    wcb = np.float32(W_c_b.astype(np.float32)[0])

    in_maps = []
    for c in range(NCORES):
        ctxT_p = np.zeros((128, int(t_off[-1])), dtype=FP8)
        ctxN_p = np.zeros((128, int(n_off[-1])), dtype=BF16)
        maskf = np.full((1, int(m_off[-1])), -1e6, dtype=np.float32)
        hidT_p = np.zeros((128, KH * BC), dtype=np.float32)
        for b in range(BC):
            gb = int(order[b * NCORES + c])
            nb = int(n[gb])
            sc = scs[b]
            idx = np.flatnonzero(~mask[gb, :, 0])
            ctxf = np.zeros((sc, DP), dtype=np.float32)
            ctxf[:nb, :D] = context[gb][idx]
            # d-major fp8: [p, t_off[b] + kd*sc + s] = ctx[s, kd*128+p]
            ctxT_p[:, int(t_off[b]) : int(t_off[b + 1])] = (
                ctxf.T.astype(FP8).reshape(KD, 128, sc)
                .transpose(1, 0, 2).reshape(128, KD * sc)
            )
            # s-major bf16: [p, n_off[b] + st*DP + d] = ctx[st*128+p, d]
            ctxnb = np.zeros((kscs[b] * 128, DP), dtype=BF16)
            ctxnb[:sc] = ctxf.astype(BF16)
            ctxN_p[:, int(n_off[b]) : int(n_off[b + 1])] = (
                ctxnb.reshape(kscs[b], 128, DP).transpose(1, 0, 2)
                .reshape(128, kscs[b] * DP)
            )
            maskf[0, int(m_off[b]) : int(m_off[b]) + nb] = wcb
            hidT_p[:, [h * BC + b for h in range(KH)]] = hip[gb].reshape(KH, 128).T
        in_maps.append({
            "ctxT": ctxT_p,
            "ctxN": ctxN_p,
            "waT": waT_p,
            "dwT": dwT_p,
            "hidT": hidT_p,
            "wcT": wcT_p,
            "maskv": maskf,
            "dbias": dbias_p,
        })
    return in_maps


def kernel(decoder_hidden, context, mask, W_a, W_b, W_c_w, W_c_b,
           dense_w, dense_b, _trace=False):
    global _GRAPH, _GRAPH_KEY
    decoder_hidden = np.asarray(decoder_hidden)
    context = np.asarray(context)
    mask = np.asarray(mask)
    W_a = np.asarray(W_a)
    W_b = np.asarray(W_b)
    W_c_w = np.asarray(W_c_w)
    W_c_b = np.asarray(W_c_b)
    dense_w = np.asarray(dense_w)
    dense_b = np.asarray(dense_b)

    n, order, scs, kscs = _plan(mask)
    key = (tuple(scs), tuple(kscs))
    if _GRAPH is None or _GRAPH_KEY != key:
        _GRAPH = _build_graph(scs, kscs)
        _GRAPH_KEY = key
    in_maps = _prep_inputs(
        decoder_hidden, context, mask, W_a, W_b, W_c_w, W_c_b,
        dense_w, dense_b, n, order, scs, kscs,
    )
    try:
        res = run_bass_kernel_spmd(
            _GRAPH, in_maps, list(range(NCORES)), trace=_trace
        )
    except Exception:
        # transient NRT/device hiccups happen occasionally; retry once
        import time as _time
        _time.sleep(2)
        res = run_bass_kernel_spmd(
            _GRAPH, in_maps, list(range(NCORES)), trace=_trace
        )
    out = np.zeros((B, H), dtype=np.float32)
    for c in range(NCORES):
        r = res.results[c]["out"]  # [128, 256] f32
        for b in range(BC):
            gb = int(order[b * NCORES + c])
            full = np.concatenate([r[32 * q + b] for q in range(4)])[:H]
            out[gb] = full
    if _trace:
        kernel.last_exec_time_ns = res.exec_time_ns
    return out.reshape(B, 1, H).astype(np.float32)


# revision 14
# speedup vs baseline: 1.0528x; 1.0528x over previous
"""Trainium2 Bass kernel for a Bahdanau-style attention module.

Reference computation (B=64, S=512, H=1000, D=2H=2000):
    ci   = context @ W_a.T                        # (B,S,H)
    hi   = decoder_hidden @ W_b.T                 # (1,B,H) -> (B,1,H)
    att  = tanh(ci + hi) @ W_c_w.T + W_c_b        # (B,S,1)
    att  = where(mask, -1e6, att); att = softmax(att, axis=1)
    ws   = att.T @ context                        # (B,1,2H)
    out  = ws @ dense_w.T + dense_b               # (B,1,H)

Strategy: data-parallel over batch across 8 NeuronCores (8 batch slots
per core, weights replicated; no collectives).  Host-side prep:
  * masked source positions are compacted away (their softmax weight is
    exactly 0), batches are sorted by unmasked count and assigned so
    that slot k holds ranks [8k, 8k+8) -- every core's slot-k matmuls
    then share one tight width SC_k instead of a global worst-case pad;
  * hidden_in = decoder_hidden @ W_b.T is computed on host (tiny GEMM)
    and shipped as the tanh bias directly;
  * all tensors are pre-cast (ctx/W_a fp8 for the DoubleRow score path,
    bf16 for the value path) and packed partition-major so every DMA is
    one contiguous 128-partition transfer.
On-chip: fp8 DoubleRow matmuls for ci, bf16 for scores/ws/dense;
softmax normalization is deferred past the weighted sum (ws accumulates
raw exp() and is rescaled once by 1/sum per batch row); the dense tail
runs as 4 concurrent PE column-quadrants.
"""

import numpy as np
import ml_dtypes

import concourse.bass as bass
import concourse.tile as tile
from concourse import bacc, mybir
from concourse.bass_utils import run_bass_kernel_spmd
from concourse.masks import make_identity

BF16 = ml_dtypes.bfloat16
FP8 = ml_dtypes.float8_e4m3
WA_SCALE = 64.0

B = 64          # global batch
BC = 8          # batch slots per core
NCORES = 8
S = 512         # encoder source length
H = 1000
HP = 1024       # padded hidden
D = 2000
DP = 2048       # padded 2*hidden
KD = DP // 128  # 16 k-tiles over padded contraction dim
KH = HP // 128  # 8 h-tiles
F32 = mybir.dt.float32
BF = mybir.dt.bfloat16
F8 = mybir.dt.float8e4

N_WARM = 3      # PE warmup matmuls (data-arrival jitter buffer)


def _build_graph(scs, kscs):
    """scs[k] = compacted source width of slot k (multiple of 32, sorted
    descending); kscs[k] = number of 128-row s-tiles for the ws path."""
    nc = bacc.Bacc()

    t_off = np.cumsum([0] + [KD * sc for sc in scs])
    n_off = np.cumsum([0] + [ksc * DP for ksc in kscs])
    m_off = np.cumsum([0] + list(scs))

    ctxT = nc.declare_dram_parameter("ctxT", [128, int(t_off[-1])], F8, isOutput=False)
    ctxN = nc.declare_dram_parameter("ctxN", [128, int(n_off[-1])], BF, isOutput=False)
    waT = nc.declare_dram_parameter("waT", [128, KH, KD, 128], F8, isOutput=False)
    dwT = nc.declare_dram_parameter("dwT", [128, KD * HP], BF, isOutput=False)
    hidT = nc.declare_dram_parameter("hidT", [128, KH * BC], F32, isOutput=False)
    wcT = nc.declare_dram_parameter("wcT", [128, KH], BF, isOutput=False)
    maskv = nc.declare_dram_parameter("maskv", [1, int(m_off[-1])], F32, isOutput=False)
    dbias = nc.declare_dram_parameter("dbias", [128, 256], F32, isOutput=False)
    out_ext = nc.declare_dram_parameter("out", [128, 256], F32, isOutput=True)

    with tile.TileContext(nc) as tc:
        with (
            tc.tile_pool(name="const", bufs=1) as cpool,
            tc.tile_pool(name="ctxTp", bufs=3) as ctxT_pool,
            tc.tile_pool(name="ctxNp", bufs=2) as ctxN_pool,
            tc.tile_pool(name="tanhp", bufs=9) as tanh_pool,
            tc.tile_pool(name="smallp", bufs=2) as small_pool,
            tc.tile_pool(name="attp", bufs=2) as att_pool,
            tc.tile_pool(name="ci", bufs=3, space="PSUM") as ci_pool,
            tc.tile_pool(name="scps", bufs=3, space="PSUM") as sc_pool,
            tc.tile_pool(name="wsps", bufs=1, space="PSUM") as ws_pool,
            tc.tile_pool(name="accps", bufs=1, space="PSUM") as acc_pool,
        ):
            # ---- PE warmup: very first ops in the graph -----------------
            # Startup is DMA-bound and the HAM clock-gate needs ~3.4us of
            # PE activity to reach 2.4 GHz.  DVE memset (fast engine
            # start) rather than gpsimd so the first matmul issues early.
            warm_sb = cpool.tile([128, 512], BF, tag="warm")
            nc.vector.memset(warm_sb[:], 0.0)
            warm_ps = acc_pool.tile([128, 512], F32, tag="acc")
            for _w in range(N_WARM):
                nc.tensor.matmul(
                    warm_ps[:],
                    warm_sb[:, 0:128],
                    warm_sb[:],
                    start=True,
                    stop=True,
                    skip_group_check=True,
                )
            warm_out = cpool.tile([1, 16], F32, tag="warmout")
            nc.vector.tensor_copy(warm_out[:], warm_ps[0:1, 0:16])
            act_prime = cpool.tile([1, 1], BF, tag="actprime")
            nc.scalar.activation(
                act_prime[:], warm_sb[0:1, 0:1],
                mybir.ActivationFunctionType.Tanh,
            )

            # ---- resident weights / constants -------------------------------
            # Startup-critical DMA order: slot-0 context + the first W_a
            # h-chunk unblock the first ci matmuls; hidT must beat the
            # first tanh; slot-1 context prefetches right behind W_a.
            ctxT_tiles = {}

            def issue_ctxT(b, gate_src=None):
                t = ctxT_pool.tile([128, KD, scs[b]], F8, tag="ctxT")
                if gate_src is not None:
                    nc.vector.tensor_copy(t[0:1, 0, 0:1], gate_src)
                nc.sync.dma_start(t[:], ctxT[:, int(t_off[b]) : int(t_off[b + 1])])
                ctxT_tiles[b] = t

            # In-flight DMAs share the HBM pipes round-robin, so an
            # unconstrained preamble starves the first ci matmul's inputs
            # (waT h0 + ctxT slot 0).  Every other early transfer is
            # WAW-gated: a 1-element DVE copy into its destination that
            # reads a predecessor's landed data, so the chain streams
            # in exactly the order the compute consumes it.
            waT_sb = cpool.tile([128, KH, KD, 128], F8, tag="waT")
            nc.sync.dma_start(waT_sb[:, 0], waT[:, 0])
            issue_ctxT(0)
            ctxT0_t = ctxT_tiles[0]

            def gate(dst_1elem, src_1elem):
                nc.vector.tensor_copy(dst_1elem, src_1elem)

            hidT_sb = cpool.tile([128, KH * BC], F32, tag="hidT")
            gate(hidT_sb[0:1, 0:1], waT_sb[0:1, 0, 0, 0:1])
            nc.scalar.dma_start(hidT_sb[:], hidT[:])
            for h in range(1, KH):
                prev = ctxT0_t[0:1, 0, 0:1] if h == 1                     else waT_sb[0:1, h - 1, 0, 0:1]
                gate(waT_sb[0:1, h, 0, 0:1], prev)
                nc.scalar.dma_start(waT_sb[:, h], waT[:, h])
            issue_ctxT(1, gate_src=waT_sb[0:1, 3, 0, 0:1])
            wcT_sb = cpool.tile([128, KH], BF, tag="wcT")
            gate(wcT_sb[0:1, 0:1], waT_sb[0:1, 4, 0, 0:1])
            nc.scalar.dma_start(wcT_sb[:], wcT[:])
            maskv_sb = cpool.tile([1, int(m_off[-1])], F32, tag="maskv")
            gate(maskv_sb[0:1, 0:1], waT_sb[0:1, 4, 0, 0:1])
            nc.scalar.dma_start(maskv_sb[:], maskv[:])
            dwT_sb = cpool.tile([128, KD * HP], BF, tag="dwT")
            dbias_sb = cpool.tile([128, 256], F32, tag="dbias")

            ident_b = cpool.tile([128, 128], BF, tag="identb")
            make_identity(nc, ident_b[:])
            one_f = cpool.tile([1, 1], F32, tag="onef")
            nc.gpsimd.memset(one_f[:], 1.0)

            # 1/sum(exp) per slot, replicated to columns {b, 32+b, 64+b,
            # 96+b} at the tail so one PE transpose yields the per-row
            # rescale vector for the ws PSUM.
            invrow = cpool.tile([1, 128], F32, tag="invrow")
            wsT_sb = cpool.tile([128, KD * BC], BF, tag="wsT")

            # ---- main pipeline over batch slots -----------------------------
            ctxN_tiles = [None] * BC
            exp_tiles = [None] * BC
            ws_psum = ws_pool.tile([128, 512], F32, tag="wsps")

            def stage_scores(b):
                """ci matmuls + tanh + scores + masked exp for slot b."""
                sc = scs[b]
                if b + 1 < BC and b >= 1:
                    issue_ctxT(b + 1)
                ctxT_t = ctxT_tiles.pop(b)
                ctxN_t = ctxN_pool.tile([128, kscs[b] * DP], BF, tag="ctxN")
                if b == 0:
                    gate(ctxN_t[0:1, 0:1], waT_sb[0:1, 5, 0, 0:1])
                nc.sync.dma_start(
                    ctxN_t[:], ctxN[:, int(n_off[b]) : int(n_off[b + 1])]
                )
                ctxN_tiles[b] = ctxN_t
                if 2 <= b <= 6:
                    # tail-only data, streamed mid-loop in 5 chunks
                    c5 = b - 2
                    seg = KD * HP // 5
                    lo = c5 * seg
                    hi = KD * HP if c5 == 4 else (c5 + 1) * seg
                    nc.sync.dma_start(dwT_sb[:, lo:hi], dwT[:, lo:hi])
                if b == 6:
                    nc.sync.dma_start(dbias_sb[:], dbias[:])

                psum_sc = sc_pool.tile([1, sc], F32, tag="sc")
                tanh_tiles = {}
                ci_tiles = {}

                def emit_ci(h):
                    psum_ci = ci_pool.tile([128, sc], F32, tag="ci")
                    for g in range(KD // 2):
                        nc.tensor.matmul(
                            psum_ci[:],
                            waT_sb[:, h, 2 * g : 2 * g + 2, :],
                            ctxT_t[:, 2 * g : 2 * g + 2, :],
                            start=(g == 0),
                            stop=(g == KD // 2 - 1),
                            perf_mode=mybir.MatmulPerfMode.DoubleRow,
                        )
                    ci_tiles[h] = psum_ci

                def emit_tanh(h):
                    tanh_t = tanh_pool.tile([128, sc], BF, tag="tanh")
                    nc.scalar.activation(
                        tanh_t[:],
                        ci_tiles.pop(h)[:],
                        mybir.ActivationFunctionType.Tanh,
                        bias=hidT_sb[:, h * BC + b : h * BC + b + 1],
                        scale=1.0 / WA_SCALE,
                    )
                    tanh_tiles[h] = tanh_t

                # tanh lags ci by `lag` h-blocks so the PE never waits on
                # ACT; all scores matmuls go after the ci blocks so the
                # big-matmul pipeline breaks once per slot.
                lag = 2 if b == 0 else 1
                for h in range(KH):
                    emit_ci(h)
                    if h >= lag:
                        emit_tanh(h - lag)
                for h in range(KH - lag, KH):
                    emit_tanh(h)
                for h in range(KH):
                    nc.tensor.matmul(
                        psum_sc[:],
                        wcT_sb[:, h : h + 1],
                        tanh_tiles.pop(h)[:],
                        start=(h == 0),
                        stop=(h == KH - 1),
                    )

                # masked exp on a single partition.  No max-subtraction:
                # scores are O(1) and masked entries are -1e6 (exp -> 0).
                # Normalization is deferred: ws accumulates raw exp and is
                # rescaled by 1/esum at the tail.
                sc_sb = small_pool.tile([1, sc], F32, tag="scsb")
                nc.vector.tensor_tensor(
                    sc_sb[:], psum_sc[:],
                    maskv_sb[0:1, int(m_off[b]) : int(m_off[b + 1])],
                    op=mybir.AluOpType.add,
                )
                exp_sb = small_pool.tile([1, sc], BF, tag="exp")
                esum = small_pool.tile([1, 1], F32, tag="esum")
                nc.scalar.activation(
                    exp_sb[:], sc_sb[:], mybir.ActivationFunctionType.Exp,
                    bias=0.0, scale=1.0, accum_out=esum[:],
                )
                nc.vector.reciprocal(invrow[0:1, b : b + 1], esum[:])
                exp_tiles[b] = exp_sb

            def stage_ws(b):
                """exp transpose + weighted sum for slot b (accumulates into
                the persistent ws psum; rows j != b add exactly zero because
                attT_b is zero outside column b)."""
                sc = scs[b]
                ksc = kscs[b]
                exp_sb = exp_tiles[b]
                attT_b = att_pool.tile([128, kscs[0] * BC], BF, tag="attTb")
                nc.gpsimd.memset(attT_b[:], 0.0)
                for st in range(ksc):
                    w = min(128, sc - st * 128)
                    pt = sc_pool.tile([128, 1], BF, tag="sc")
                    nc.tensor.transpose(
                        pt[0:w, :], exp_sb[0:1, st * 128 : st * 128 + w],
                        ident_b[0:1, 0:1],
                    )
                    nc.vector.tensor_copy(
                        attT_b[0:w, st * BC + b : st * BC + b + 1],
                        pt[0:w, :],
                    )
                ctxN_t = ctxN_tiles[b]
                for st in range(ksc):
                    for nch in range(4):
                        nc.tensor.matmul(
                            ws_psum[32 * nch : 32 * nch + BC, :],
                            attT_b[:, st * BC : (st + 1) * BC],
                            ctxN_t[:, st * DP + nch * 512 : st * DP + (nch + 1) * 512],
                            start=(b == 0 and st == 0),
                            stop=(b == BC - 1 and st == ksc - 1),
                            tile_position=(0, 32 * nch),
                            skip_group_check=True,
                        )

            # software pipeline: scores(b) runs while ws(b-1) consumes
            for b in range(BC + 1):
                if b < BC:
                    stage_scores(b)
                if b >= 1:
                    stage_ws(b - 1)

            # ---- tail: normalize ws, transpose, dense layer -----------------
            for n in range(1, 4):
                nc.vector.tensor_copy(
                    invrow[0:1, 32 * n : 32 * n + BC], invrow[0:1, 0:BC]
                )
            einv_ps = sc_pool.tile([128, 1], F32, tag="sc")
            nc.tensor.transpose(einv_ps[:], invrow[:], one_f[:])
            einv_sb = cpool.tile([128, 1], F32, tag="einvsb")
            nc.vector.tensor_copy(einv_sb[:], einv_ps[:])

            ws_col = cpool.tile([128, 512], BF, tag="wscol")
            nc.vector.tensor_scalar_mul(ws_col[:], ws_psum[:], einv_sb[:])
            # consecutive transposes hit different PE row-groups so they
            # overlap in the array
            for k in [4 * q + j for j in range(4) for q in range(4)]:
                nch, kk = divmod(k, 4)
                pt = sc_pool.tile([128, BC], BF, tag="sc")
                nc.tensor.transpose(
                    pt[:],
                    ws_col[32 * nch : 32 * nch + BC, kk * 128 : (kk + 1) * 128],
                    ident_b[32 * nch : 32 * nch + BC, 32 * nch : 32 * nch + BC],
                    tile_position=(32 * nch, 0),
                )
                nc.vector.tensor_copy(wsT_sb[:, k * BC : (k + 1) * BC], pt[:])

            # dense: 4 concurrent column-quadrants, h-quarter q -> rows 32q+b
            psum_d = acc_pool.tile([128, 256], F32, tag="acc")
            for k in range(KD):
                for q in range(4):
                    nc.tensor.matmul(
                        psum_d[32 * q : 32 * q + BC, :],
                        wsT_sb[:, k * BC : (k + 1) * BC],
                        dwT_sb[:, k * HP + q * 256 : k * HP + (q + 1) * 256],
                        start=(k == 0),
                        stop=(k == KD - 1),
                        tile_position=(0, 32 * q),
                        skip_group_check=True,
                    )
            out_sb = cpool.tile([128, 256], F32, tag="outsb")
            nc.vector.tensor_tensor(
                out_sb[:], psum_d[:], dbias_sb[:], op=mybir.AluOpType.add
            )
            nc.sync.dma_start(out_ext[:], out_sb[:])

    nc.compile()
    return nc


_GRAPH = None
_GRAPH_KEY = None


def _plan(mask):
    """Sorted slot assignment + per-slot compact widths."""
    n = (~mask[:, :, 0]).sum(axis=1).astype(np.int64)
    order = np.argsort(-n, kind="stable")
    scs, kscs = [], []
    for k in range(BC):
        m = int(n[order[k * NCORES]])
        sc = max(32, -(-m // 4) * 4)
        scs.append(sc)
        kscs.append(-(-sc // 128))
    return n, order, scs, kscs


def _prep_inputs(decoder_hidden, context, mask, W_a, W_b, W_c_w, W_c_b,
                 dense_w, dense_b, n, order, scs, kscs):
    """Shard + pad + cast + pack all inputs into per-core input maps."""
    t_off = np.cumsum([0] + [KD * sc for sc in scs])
    n_off = np.cumsum([0] + [ksc * DP for ksc in kscs])
    m_off = np.cumsum([0] + list(scs))

    # W_a, h-major fp8: [p, h, kd, c] = W_a.T[kd*128+p, h*128+c] * WA_SCALE
    wa = np.zeros((DP, HP), dtype=np.float32)
    wa[:D, :H] = W_a.T.astype(np.float32) * WA_SCALE
    waT_p = np.ascontiguousarray(
        wa.reshape(KD, 128, KH, 128).transpose(1, 2, 0, 3)
    ).astype(FP8)

    dw = np.zeros((DP, HP), dtype=BF16)
    dw[:D, :H] = dense_w.T.astype(BF16)
    dwT_p = np.ascontiguousarray(
        dw.reshape(KD, 128, HP).transpose(1, 0, 2).reshape(128, KD * HP)
    )
    wc = np.zeros((HP,), dtype=np.float32)
    wc[:H] = W_c_w[0].astype(np.float32)
    wcT_p = np.ascontiguousarray(wc.reshape(KH, 128).T.astype(BF16))

    db = np.zeros((HP,), dtype=np.float32)
    db[:H] = dense_b.astype(np.float32)
    dbias_p = np.zeros((128, 256), dtype=np.float32)
    for q in range(4):
        dbias_p[32 * q : 32 * q + BC, :] = db[q * 256 : (q + 1) * 256]

    # hidden_in on host (tiny GEMM), padded + partition-major
    hi = decoder_hidden[0].astype(np.float32) @ W_b.T.astype(np.float32)  # (B, H)
    hip = np.zeros((B, HP), dtype=np.float32)
    hip[:, :H] = hi

    wcb = np.float32(W_c_b.astype(np.float32)[0])

    in_maps = []
    for c in range(NCORES):
        ctxT_p = np.zeros((128, int(t_off[-1])), dtype=FP8)
        ctxN_p = np.zeros((128, int(n_off[-1])), dtype=BF16)
        maskf = np.full((1, int(m_off[-1])), -1e6, dtype=np.float32)
        hidT_p = np.zeros((128, KH * BC), dtype=np.float32)
        for b in range(BC):
            gb = int(order[b * NCORES + c])
            nb = int(n[gb])
            sc = scs[b]
            idx = np.flatnonzero(~mask[gb, :, 0])
            ctxf = np.zeros((sc, DP), dtype=np.float32)
            ctxf[:nb, :D] = context[gb][idx]
            # d-major fp8: [p, t_off[b] + kd*sc + s] = ctx[s, kd*128+p]
            ctxT_p[:, int(t_off[b]) : int(t_off[b + 1])] = (
                ctxf.T.astype(FP8).reshape(KD, 128, sc)
                .transpose(1, 0, 2).reshape(128, KD * sc)
            )
            # s-major bf16: [p, n_off[b] + st*DP + d] = ctx[st*128+p, d]
            ctxnb = np.zeros((kscs[b] * 128, DP), dtype=BF16)
            ctxnb[:sc] = ctxf.astype(BF16)
            ctxN_p[:, int(n_off[b]) : int(n_off[b + 1])] = (
                ctxnb.reshape(kscs[b], 128, DP).transpose(1, 0, 2)
                .reshape(128, kscs[b] * DP)
            )
            maskf[0, int(m_off[b]) : int(m_off[b]) + nb] = wcb
            hidT_p[:, [h * BC + b for h in range(KH)]] = hip[gb].reshape(KH, 128).T
        in_maps.append({
            "ctxT": ctxT_p,
            "ctxN": ctxN_p,
            "waT": waT_p,
            "dwT": dwT_p,
            "hidT": hidT_p,
            "wcT": wcT_p,
            "maskv": maskf,
            "dbias": dbias_p,
        })
    return in_maps


def kernel(decoder_hidden, context, mask, W_a, W_b, W_c_w, W_c_b,
           dense_w, dense_b, _trace=False):
    global _GRAPH, _GRAPH_KEY
    decoder_hidden = np.asarray(decoder_hidden)
    context = np.asarray(context)
    mask = np.asarray(mask)
    W_a = np.asarray(W_a)
    W_b = np.asarray(W_b)
    W_c_w = np.asarray(W_c_w)
    W_c_b = np.asarray(W_c_b)
    dense_w = np.asarray(dense_w)
    dense_b = np.asarray(dense_b)

    n, order, scs, kscs = _plan(mask)
    key = (tuple(scs), tuple(kscs))
    if _GRAPH is None or _GRAPH_KEY != key:
        _GRAPH = _build_graph(scs, kscs)
        _GRAPH_KEY = key
    in_maps = _prep_inputs(
        decoder_hidden, context, mask, W_a, W_b, W_c_w, W_c_b,
        dense_w, dense_b, n, order, scs, kscs,
    )
    try:
        res = run_bass_kernel_spmd(
            _GRAPH, in_maps, list(range(NCORES)), trace=_trace
        )
    except Exception:
        # transient NRT/device hiccups happen occasionally; retry once
        import time as _time
        _time.sleep(2)
        res = run_bass_kernel_spmd(
            _GRAPH, in_maps, list(range(NCORES)), trace=_trace
        )
    out = np.zeros((B, H), dtype=np.float32)
    for c in range(NCORES):
        r = res.results[c]["out"]  # [128, 256] f32
        for b in range(BC):
            gb = int(order[b * NCORES + c])
            full = np.concatenate([r[32 * q + b] for q in range(4)])[:H]
            out[gb] = full
    if _trace:
        kernel.last_exec_time_ns = res.exec_time_ns
    return out.reshape(B, 1, H).astype(np.float32)
